# revision 19
# baseline (speedup 1.0000x reference)
"""Trainium2 kernel for nn_KernelizedAttention_14869176779022.

Math note: the reference computes
    out = (s * v) / s        with s = <phi_q, phi_k> > 0  (sums of exps)
so out == v == x @ Wv.T + bv exactly (up to one multiply/divide rounding).
The kernel therefore only computes the Wv linear layer.

Sharding: data-parallel over the 8192 (B*S) positions - 1024 rows per core.
Wv (pre-transposed) and bv are replicated. x is pre-swizzled on the host into
the exact SBUF layout the TensorEngine wants (contraction dim on partitions).

Schedule (measured on HW): all engines are blocked by a ~6.8us framework
preamble; first DMA bytes land ~8.2us (trace time). Aggregate HBM rate under
8-core SPMD is ~270 GB/s split over the active queues. The PE floor for the
per-core 1024^3 bf16 GEMM is 128 MMs x ~215ns = 27.5us warm. So: warm the PE
with dummy matmuls during the load lead-in (HAM clock-gate releases after
~3.4us of sustained busy), stream inputs in consumption order across the two
HWDGE rings + the SWDGE queue, and chase the stream with an A-pass (n-cols
0:512) in m-pair/k-outer order followed by a dense B-pass (cols 512:1024).
Output is stored as bf16 (halves store bytes; host upcasts; adds ~1e-3 fro
error against a 2e-2 budget).
"""

import sys

if "/opt/trn_rl_repo" not in sys.path:
    sys.path.insert(0, "/opt/trn_rl_repo")

import numpy as np

B, S, E = 2, 4096, 1024
N_CORES = 8
ROWS = B * S            # 8192
R = ROWS // N_CORES     # 1024 rows per core
P = 128                 # partitions
KT = E // P             # 8 contraction tiles
MT = R // P             # 8 row tiles per core
NH = 2                  # n-half passes (512 output cols each)
NSZ = E // NH           # 512 = one PSUM bank (fp32)

_NC_CACHE = {}


def _build_nc(**bass_kwargs):
    import concourse.bass as bass
    import concourse.mybir as mybir
    from concourse import bacc
    from concourse.tile import TileContext

    f32 = mybir.dt.float32
    bf16 = mybir.dt.bfloat16
    nc = bacc.Bacc(None, target_bir_lowering=False, **bass_kwargs)

    fp8 = mybir.dt.float8e3
    # xb[p, (m*KT + k)*P + mm] = x_shard[m*P + mm, k*P + p]  (e3m4, host-packed)
    xb = nc.dram_tensor("xb", [P, MT * KT * P], fp8, kind="ExternalInput")
    # wv[p, (h*KT + k)*NSZ + c] = 64*Wv[h*NSZ + c, k*P + p]   (e3m4, host-packed;
    # x64 scale keeps Wv ~N(0,1/32) inside e3m4's normal range; undone on host)
    wv = nc.dram_tensor("wv", [P, NH * KT * NSZ], fp8, kind="ExternalInput")
    # bias pre-broadcast to 128 partitions, x64 to match the wv scale (bf16)
    bvb = nc.dram_tensor("bvb", [P, E], bf16, kind="ExternalInput")
    out = nc.dram_tensor("out", [R, E], bf16, kind="ExternalOutput")

    with TileContext(nc) as tc:
        with (
            tc.tile_pool(name="consts", bufs=1) as consts,
            tc.tile_pool(name="xpool", bufs=1) as xpool,
            tc.tile_pool(name="wpool", bufs=1) as wpool,
            tc.tile_pool(name="opool", bufs=MT) as opool,
            tc.tile_pool(name="ppool", bufs=7, space="PSUM") as ppool,
            tc.tile_pool(name="dpool", bufs=1, space="PSUM") as dpool,
        ):
            # PE warm-up: dummy matmuls on a zeroed scratch tile keep the PE
            # busy from preamble-end (~7.6us) until the first input chunks
            # land (~12us), so the HAM clock-gate releases at ~11us and the
            # real matmuls all run at 2.4 GHz. Never read; costs 1 PSUM bank.
            dum_sb = consts.tile([P, NSZ], bf16, tag="dum")
            nc.gpsimd.memset(dum_sb, 0.0)
            dum_ps = dpool.tile([P, NSZ], f32, tag="dps")
            for _ in range(8):
                nc.tensor.matmul(
                    dum_ps, dum_sb[:, :P], dum_sb, start=True, stop=True
                )

            bias_sb = consts.tile([P, E], bf16, tag="bias")
            wv_sb = wpool.tile([P, NH * KT * NSZ], bf16, tag="wv")
            x_sb = xpool.tile([P, MT * KT * P], bf16, tag="x")

            # All x/wv loads go through the single SWDGE queue, which
            # casts e3m4 -> bf16 during the DMA (SWDGE-only feature). Input
            # bytes drop to 2.25MB so the whole stream lands by ~17us trace
            # time. Chunks are ordered in exact consumption order; bias rides
            # the otherwise-idle ACT ring. Stores alternate SP/ACT.
            xm = KT * P             # one m-tile of x: 128KB in e3m4
            wk = NSZ                # one k-tile of one n-half: 64KB in e3m4

            nc.gpsimd.dma_start(
                out=x_sb[:, 0 : 2 * xm], in_=xb[:, 0 : 2 * xm]
            )
            nc.gpsimd.dma_start(
                out=wv_sb[:, 0 : 2 * wk], in_=wv[:, 0 : 2 * wk]
            )
            nc.gpsimd.dma_start(
                out=wv_sb[:, 2 * wk : 4 * wk], in_=wv[:, 2 * wk : 4 * wk]
            )
            nc.gpsimd.dma_start(
                out=wv_sb[:, 4 * wk : 8 * wk], in_=wv[:, 4 * wk : 8 * wk]
            )
            nc.scalar.dma_start(out=bias_sb, in_=bvb[:, :])
            nc.gpsimd.dma_start(
                out=x_sb[:, 2 * xm : 4 * xm], in_=xb[:, 2 * xm : 4 * xm]
            )
            nc.gpsimd.dma_start(
                out=x_sb[:, 4 * xm : 6 * xm], in_=xb[:, 4 * xm : 6 * xm]
            )
            nc.gpsimd.dma_start(
                out=wv_sb[:, 8 * wk : 12 * wk], in_=wv[:, 8 * wk : 12 * wk]
            )
            nc.gpsimd.dma_start(
                out=x_sb[:, 6 * xm : 8 * xm], in_=xb[:, 6 * xm : 8 * xm]
            )
            nc.gpsimd.dma_start(
                out=wv_sb[:, 12 * wk : 16 * wk], in_=wv[:, 12 * wk : 16 * wk]
            )

            om_tiles = [
                opool.tile([P, E], bf16, name=f"om{m}", tag="om")
                for m in range(MT)
            ]

            def drain(h, m, ps):
                nc.vector.tensor_add(
                    out=om_tiles[m][:, h * NSZ : (h + 1) * NSZ],
                    in0=ps,
                    in1=bias_sb[:, h * NSZ : (h + 1) * NSZ],
                )
                if h == NH - 1:
                    dst = bass.AP(
                        tensor=out.tensor if hasattr(out, "tensor") else out,
                        offset=m * P * E,
                        ap=[[E, P], [1, E]],
                    )
                    ring = nc.sync if (m % 2 == 0) else nc.scalar
                    ring.dma_start(out=dst, in_=om_tiles[m])

            # A-pass (h=0): m-pair blocks, k-outer inside each pair, so the
            # PE chases the k-ordered wv h0 stream as chunks arrive.
            for pair in range(MT // 2):
                ma, mb = 2 * pair, 2 * pair + 1
                psa = ppool.tile([P, NSZ], f32, name=f"psa{ma}", tag="ps")
                psb = ppool.tile([P, NSZ], f32, name=f"psa{mb}", tag="ps")
                for k in range(KT):
                    for m, ps in ((ma, psa), (mb, psb)):
                        nc.tensor.matmul(
                            ps,
                            x_sb[:, (m * KT + k) * P : (m * KT + k + 1) * P],
                            wv_sb[:, k * NSZ : (k + 1) * NSZ],
                            start=(k == 0),
                            stop=(k == KT - 1),
                        )
                drain(0, ma, psa)
                drain(0, mb, psb)

            # B-pass (h=1): m-outer, wv h1 fully resident by now.
            for m in range(MT):
                ps = ppool.tile([P, NSZ], f32, name=f"psb{m}", tag="ps")
                for k in range(KT):
                    nc.tensor.matmul(
                        ps,
                        x_sb[:, (m * KT + k) * P : (m * KT + k + 1) * P],
                        wv_sb[:, (KT + k) * NSZ : (KT + k + 1) * NSZ],
                        start=(k == 0),
                        stop=(k == KT - 1),
                    )
                drain(1, m, ps)
    nc.compile()
    return nc


def _get_nc():
    if "nc" not in _NC_CACHE:
        _NC_CACHE["nc"] = _build_nc()
    return _NC_CACHE["nc"]


def _prep_in_maps(x, Wv, bv):
    import ml_dtypes

    bf16 = ml_dtypes.bfloat16
    x = np.ascontiguousarray(np.asarray(x, dtype=np.float32))
    Wv = np.asarray(Wv, dtype=np.float32)
    bv = np.asarray(bv, dtype=np.float32)

    e3m4 = ml_dtypes.float8_e3m4
    xf = x.reshape(ROWS, E)
    # wvb[p, (h*KT + k)*NSZ + c] = 64*Wv[h*NSZ + c, k*P + p]  (e3m4)
    #   [j=(h c), (k p)] -> [p, (h k c)]
    wvp = np.ascontiguousarray(
        (Wv * 64.0)
        .reshape(NH, NSZ, KT, P)
        .transpose(3, 0, 2, 1)
        .reshape(P, NH * KT * NSZ)
        .astype(e3m4)
    )
    bv2 = np.ascontiguousarray(
        np.broadcast_to((bv * 64.0).reshape(1, E), (P, E)).astype(bf16)
    )

    in_maps = []
    for c in range(N_CORES):
        xs = xf[c * R : (c + 1) * R]                    # [R, E]
        # xb[p, (m*KT+k)*P+mm] = xs[m*P+mm, k*P+p]
        xbc = np.ascontiguousarray(
            xs.reshape(MT, P, KT, P)
            .transpose(3, 0, 2, 1)
            .reshape(P, MT * KT * P)
            .astype(e3m4)
        )
        in_maps.append({"xb": xbc, "wv": wvp, "bvb": bv2})
    return in_maps


def _install_ntff_hook():
    """This image's antenv lacks axon_hooks; recreate the bridge module so
    run_bass_kernel_spmd(trace=True) can reach the ctypes NTFF profiler."""
    import types

    if "antenv.axon_hooks" in sys.modules:
        return
    try:
        from trn_agent_boot.trn_boot import _ntff_profile_via_ctypes
    except ImportError:
        return
    hook = _ntff_profile_via_ctypes("/opt/axon/libaxon_pjrt.so")
    mod = types.ModuleType("antenv.axon_hooks")
    mod._hook = hook
    mod.get_axon_ntff_profile_hook = lambda: mod._hook
    mod.set_axon_ntff_profile_hook = lambda h: setattr(mod, "_hook", h)
    sys.modules["antenv.axon_hooks"] = mod


def _run(x, Wv, bv, trace=False):
    from concourse.bass_utils import run_bass_kernel_spmd

    if trace:
        _install_ntff_hook()
    nc = _get_nc()
    in_maps = _prep_in_maps(x, Wv, bv)
    res = run_bass_kernel_spmd(
        nc, in_maps, core_ids=list(range(N_CORES)), trace=trace
    )
    out = np.concatenate(
        [np.asarray(res.results[c]["out"]) for c in range(N_CORES)], axis=0
    )
    return out.reshape(B, S, E).astype(np.float32) * (1.0 / 64.0), res


def kernel(x, Wq, bq, Wk, bk, Wv, bv, weights):
    out, _ = _run(x, Wv, bv, trace=False)
    return out


def kernel_traced(x, Wq, bq, Wk, bk, Wv, bv, weights):
    """Like kernel() but with NTFF profiling; returns (out, BassKernelResults)."""
    out, res = _run(x, Wv, bv, trace=True)
    return out, res


# revision 20
# speedup vs baseline: 1.0117x; 1.0117x over previous
"""Trainium2 kernel for nn_KernelizedAttention_14869176779022.

Math note: the reference computes
    out = (s * v) / s        with s = <phi_q, phi_k> > 0  (sums of exps)
so out == v == x @ Wv.T + bv exactly (up to one multiply/divide rounding).
The kernel therefore only computes the Wv linear layer.

Sharding: data-parallel over the 8192 (B*S) positions - 1024 rows per core.
Wv (pre-transposed) and bv are replicated. x is pre-swizzled on the host into
the exact SBUF layout the TensorEngine wants (contraction dim on partitions).

Schedule (measured on HW): all engines are blocked by a ~6.8us framework
preamble; first DMA bytes land ~8.2us (trace time). Aggregate HBM rate under
8-core SPMD is ~270 GB/s split over the active queues. The PE floor for the
per-core 1024^3 bf16 GEMM is 128 MMs x ~215ns = 27.5us warm. So: warm the PE
with dummy matmuls during the load lead-in (HAM clock-gate releases after
~3.4us of sustained busy), stream inputs in consumption order across the two
HWDGE rings + the SWDGE queue, and chase the stream with an A-pass (n-cols
0:512) in m-pair/k-outer order followed by a dense B-pass (cols 512:1024).
Output is stored as bf16 (halves store bytes; host upcasts; adds ~1e-3 fro
error against a 2e-2 budget).
"""

import sys

if "/opt/trn_rl_repo" not in sys.path:
    sys.path.insert(0, "/opt/trn_rl_repo")

import numpy as np

B, S, E = 2, 4096, 1024
N_CORES = 8
ROWS = B * S            # 8192
R = ROWS // N_CORES     # 1024 rows per core
P = 128                 # partitions
KT = E // P             # 8 contraction tiles
MT = R // P             # 8 row tiles per core
NH = 2                  # n-half passes (512 output cols each)
NSZ = E // NH           # 512 = one PSUM bank (fp32)

_NC_CACHE = {}


def _build_nc(**bass_kwargs):
    import concourse.bass as bass
    import concourse.mybir as mybir
    from concourse import bacc
    from concourse.tile import TileContext

    f32 = mybir.dt.float32
    bf16 = mybir.dt.bfloat16
    nc = bacc.Bacc(None, target_bir_lowering=False, **bass_kwargs)

    fp8 = mybir.dt.float8e3
    # xb[p, (m*KT + k)*P + mm] = x_shard[m*P + mm, k*P + p]  (e3m4, host-packed)
    xb = nc.dram_tensor("xb", [P, MT * KT * P], fp8, kind="ExternalInput")
    # wv[p, (h*KT + k)*NSZ + c] = 64*Wv[h*NSZ + c, k*P + p]   (e3m4, host-packed;
    # x64 scale keeps Wv ~N(0,1/32) inside e3m4's normal range; undone on host)
    wv = nc.dram_tensor("wv", [P, NH * KT * NSZ], fp8, kind="ExternalInput")
    # bias pre-broadcast to 128 partitions, x64 to match the wv scale (bf16)
    bvb = nc.dram_tensor("bvb", [P, E], bf16, kind="ExternalInput")
    # bf16 head copies for the latency-critical first chunks (HWDGE rings
    # cannot cast, and the SWDGE stream's first completions land ~14us)
    xh = nc.dram_tensor("xh", [P, KT * P], bf16, kind="ExternalInput")
    wh = nc.dram_tensor("wh", [P, NSZ], bf16, kind="ExternalInput")
    out = nc.dram_tensor("out", [R, E], bf16, kind="ExternalOutput")

    with TileContext(nc) as tc:
        with (
            tc.tile_pool(name="consts", bufs=1) as consts,
            tc.tile_pool(name="xpool", bufs=1) as xpool,
            tc.tile_pool(name="wpool", bufs=1) as wpool,
            tc.tile_pool(name="opool", bufs=MT) as opool,
            tc.tile_pool(name="ppool", bufs=7, space="PSUM") as ppool,
            tc.tile_pool(name="dpool", bufs=1, space="PSUM") as dpool,
        ):
            # PE warm-up: dummy matmuls on a zeroed scratch tile keep the PE
            # busy from preamble-end (~7.6us) until the first input chunks
            # land (~12us), so the HAM clock-gate releases at ~11us and the
            # real matmuls all run at 2.4 GHz. Never read; costs 1 PSUM bank.
            dum_sb = consts.tile([P, NSZ], bf16, tag="dum")
            nc.gpsimd.memset(dum_sb, 0.0)
            dum_ps = dpool.tile([P, NSZ], f32, tag="dps")
            for _ in range(6):
                nc.tensor.matmul(
                    dum_ps, dum_sb[:, :P], dum_sb, start=True, stop=True
                )

            bias_sb = consts.tile([P, E], bf16, tag="bias")
            wv_sb = wpool.tile([P, NH * KT * NSZ], bf16, tag="wv")
            x_sb = xpool.tile([P, MT * KT * P], bf16, tag="x")

            # All x/wv loads go through the single SWDGE queue, which
            # casts e3m4 -> bf16 during the DMA (SWDGE-only feature). Input
            # bytes drop to 2.25MB so the whole stream lands by ~17us trace
            # time. Chunks are ordered in exact consumption order; bias rides
            # the otherwise-idle ACT ring. Stores alternate SP/ACT.
            xm = KT * P             # one m-tile of x: 128KB in e3m4
            wk = NSZ                # one k-tile of one n-half: 64KB in e3m4

            nc.sync.dma_start(out=x_sb[:, 0:xm], in_=xh[:, :])
            nc.scalar.dma_start(out=wv_sb[:, 0:wk], in_=wh[:, :])
            nc.gpsimd.dma_start(
                out=x_sb[:, xm : 2 * xm], in_=xb[:, xm : 2 * xm]
            )
            nc.gpsimd.dma_start(
                out=wv_sb[:, wk : 4 * wk], in_=wv[:, wk : 4 * wk]
            )
            nc.gpsimd.dma_start(
                out=wv_sb[:, 4 * wk : 8 * wk], in_=wv[:, 4 * wk : 8 * wk]
            )
            nc.scalar.dma_start(out=bias_sb, in_=bvb[:, :])
            nc.gpsimd.dma_start(
                out=x_sb[:, 2 * xm : 4 * xm], in_=xb[:, 2 * xm : 4 * xm]
            )
            nc.gpsimd.dma_start(
                out=x_sb[:, 4 * xm : 6 * xm], in_=xb[:, 4 * xm : 6 * xm]
            )
            nc.gpsimd.dma_start(
                out=wv_sb[:, 8 * wk : 12 * wk], in_=wv[:, 8 * wk : 12 * wk]
            )
            nc.gpsimd.dma_start(
                out=x_sb[:, 6 * xm : 8 * xm], in_=xb[:, 6 * xm : 8 * xm]
            )
            nc.gpsimd.dma_start(
                out=wv_sb[:, 12 * wk : 16 * wk], in_=wv[:, 12 * wk : 16 * wk]
            )

            om_tiles = [
                opool.tile([P, E], bf16, name=f"om{m}", tag="om")
                for m in range(MT)
            ]

            def drain(h, m, ps):
                nc.vector.tensor_add(
                    out=om_tiles[m][:, h * NSZ : (h + 1) * NSZ],
                    in0=ps,
                    in1=bias_sb[:, h * NSZ : (h + 1) * NSZ],
                )
                if h == NH - 1:
                    if m == MT - 1:
                        for half, ring in ((0, nc.sync), (1, nc.scalar)):
                            dst = bass.AP(
                                tensor=out.tensor if hasattr(out, "tensor") else out,
                                offset=m * P * E + half * NSZ,
                                ap=[[E, P], [1, NSZ]],
                            )
                            ring.dma_start(
                                out=dst,
                                in_=om_tiles[m][:, half * NSZ : (half + 1) * NSZ],
                            )
                    else:
                        dst = bass.AP(
                            tensor=out.tensor if hasattr(out, "tensor") else out,
                            offset=m * P * E,
                            ap=[[E, P], [1, E]],
                        )
                        ring = nc.sync if (m % 2 == 0) else nc.scalar
                        ring.dma_start(out=dst, in_=om_tiles[m])

            # A-pass (h=0): m0 chases the k-stream solo, m1 follows dense,
            # remaining m-pairs run k-outer as their x chunks land.
            for m in (0, 1):
                ps = ppool.tile([P, NSZ], f32, name=f"psa{m}", tag="ps")
                for k in range(KT):
                    nc.tensor.matmul(
                        ps,
                        x_sb[:, (m * KT + k) * P : (m * KT + k + 1) * P],
                        wv_sb[:, k * NSZ : (k + 1) * NSZ],
                        start=(k == 0),
                        stop=(k == KT - 1),
                    )
                drain(0, m, ps)
            for pair in range(1, MT // 2):
                ma, mb = 2 * pair, 2 * pair + 1
                psa = ppool.tile([P, NSZ], f32, name=f"psa{ma}", tag="ps")
                psb = ppool.tile([P, NSZ], f32, name=f"psa{mb}", tag="ps")
                for k in range(KT):
                    for m, ps in ((ma, psa), (mb, psb)):
                        nc.tensor.matmul(
                            ps,
                            x_sb[:, (m * KT + k) * P : (m * KT + k + 1) * P],
                            wv_sb[:, k * NSZ : (k + 1) * NSZ],
                            start=(k == 0),
                            stop=(k == KT - 1),
                        )
                drain(0, ma, psa)
                drain(0, mb, psb)

            # B-pass (h=1): m-outer, wv h1 fully resident by now.
            for m in range(MT):
                ps = ppool.tile([P, NSZ], f32, name=f"psb{m}", tag="ps")
                for k in range(KT):
                    nc.tensor.matmul(
                        ps,
                        x_sb[:, (m * KT + k) * P : (m * KT + k + 1) * P],
                        wv_sb[:, (KT + k) * NSZ : (KT + k + 1) * NSZ],
                        start=(k == 0),
                        stop=(k == KT - 1),
                    )
                drain(1, m, ps)
    nc.compile()
    return nc


def _get_nc():
    if "nc" not in _NC_CACHE:
        _NC_CACHE["nc"] = _build_nc()
    return _NC_CACHE["nc"]


def _prep_in_maps(x, Wv, bv):
    import ml_dtypes

    bf16 = ml_dtypes.bfloat16
    x = np.ascontiguousarray(np.asarray(x, dtype=np.float32))
    Wv = np.asarray(Wv, dtype=np.float32)
    bv = np.asarray(bv, dtype=np.float32)

    e3m4 = ml_dtypes.float8_e3m4
    xf = x.reshape(ROWS, E)
    # wvb[p, (h*KT + k)*NSZ + c] = 64*Wv[h*NSZ + c, k*P + p]  (e3m4)
    #   [j=(h c), (k p)] -> [p, (h k c)]
    wvp = np.ascontiguousarray(
        (Wv * 64.0)
        .reshape(NH, NSZ, KT, P)
        .transpose(3, 0, 2, 1)
        .reshape(P, NH * KT * NSZ)
        .astype(e3m4)
    )
    bv2 = np.ascontiguousarray(
        np.broadcast_to((bv * 64.0).reshape(1, E), (P, E)).astype(bf16)
    )
    wh0 = np.ascontiguousarray(wvp[:, :NSZ].astype(bf16))

    in_maps = []
    for c in range(N_CORES):
        xs = xf[c * R : (c + 1) * R]                    # [R, E]
        # xb[p, (m*KT+k)*P+mm] = xs[m*P+mm, k*P+p]
        xbc = np.ascontiguousarray(
            xs.reshape(MT, P, KT, P)
            .transpose(3, 0, 2, 1)
            .reshape(P, MT * KT * P)
            .astype(e3m4)
        )
        xh = np.ascontiguousarray(xbc[:, : KT * P].astype(bf16))
        in_maps.append(
            {"xb": xbc, "wv": wvp, "bvb": bv2, "xh": xh, "wh": wh0}
        )
    return in_maps


def _install_ntff_hook():
    """This image's antenv lacks axon_hooks; recreate the bridge module so
    run_bass_kernel_spmd(trace=True) can reach the ctypes NTFF profiler."""
    import types

    if "antenv.axon_hooks" in sys.modules:
        return
    try:
        from trn_agent_boot.trn_boot import _ntff_profile_via_ctypes
    except ImportError:
        return
    hook = _ntff_profile_via_ctypes("/opt/axon/libaxon_pjrt.so")
    mod = types.ModuleType("antenv.axon_hooks")
    mod._hook = hook
    mod.get_axon_ntff_profile_hook = lambda: mod._hook
    mod.set_axon_ntff_profile_hook = lambda h: setattr(mod, "_hook", h)
    sys.modules["antenv.axon_hooks"] = mod


def _run(x, Wv, bv, trace=False):
    from concourse.bass_utils import run_bass_kernel_spmd

    if trace:
        _install_ntff_hook()
    nc = _get_nc()
    in_maps = _prep_in_maps(x, Wv, bv)
    res = run_bass_kernel_spmd(
        nc, in_maps, core_ids=list(range(N_CORES)), trace=trace
    )
    out = np.concatenate(
        [np.asarray(res.results[c]["out"]) for c in range(N_CORES)], axis=0
    )
    return out.reshape(B, S, E).astype(np.float32) * (1.0 / 64.0), res


def kernel(x, Wq, bq, Wk, bk, Wv, bv, weights):
    out, _ = _run(x, Wv, bv, trace=False)
    return out


def kernel_traced(x, Wq, bq, Wk, bk, Wv, bv, weights):
    """Like kernel() but with NTFF profiling; returns (out, BassKernelResults)."""
    out, res = _run(x, Wv, bv, trace=True)
    return out, res
